# revision 1
# baseline (speedup 1.0000x reference)
"""Trainium2 Bass kernel for NeuralSumProductModel (LDPC sum-product decoder).

Contract: kernel(**inputs) takes FULL inputs (llr [512,8192] f32,
var_index [24576] i32, chk_index [24576] i32) and returns the FULL
output [5, 512, 8192] f32, matching reference.reference().

Design (per NeuronCore, batch sharded 512 -> 8 x 64):
  - partitions = (edge-half h, batch b): p = h*64 + b
  - edges in check-major order (sorted by check, 6 per check); half h owns
    checks [h*2048,(h+1)*2048) = edge cols [h*12288,(h+1)*12288)
  - one big SBUF gather TABLE [128, 45056] f32:
      [locA 0:12288 | foreign 12288:24576 | locB 24576:36864 | x 36864:45056]
    ext state ping-pongs between locA/locB by iteration parity so gathers of
    the old state never conflict with writes of the new state. 'foreign' is
    cross-filled by DMA from the partner partition half each iteration.
  - var-side ops are GPSIMD ap_gather's: msg_e = xs_e + ext[sib1] + ext[sib2]
    (siblings share e's variable), out_v = x_v + sum of ext at v's 3 edges.
  - check-side ops are strided free-axis DVE/ACT ops over groups of 6.
  - magnitude via phi involution: ext_mag = -ln(max(tanh(-d/2), TCLIP)),
    exactly 2*atanh(min(e^d, 1-1e-7)); sign via float sign-product tree.
"""

import os
import sys

import numpy as np

for _p in ("/opt/trn_rl_repo", "/root/.axon_site/_ro/trn_rl_repo"):
    if os.path.isdir(_p) and _p not in sys.path:
        sys.path.insert(0, _p)

N_VAR, N_CHK, DV, DC = 8192, 4096, 3, 6
E = N_VAR * DV  # 24576
BATCH, N_ITER, N_CORES = 512, 5, 8
BC = BATCH // N_CORES           # 64 batch rows per core
HE = E // 2                     # 12288 edge cols per half
HC = N_CHK // 2                 # 2048 checks per half
HV = N_VAR // 2                 # 4096 vars per half
N_ECH = 16                      # check chunks per iteration
ECH = HE // N_ECH               # 768 edge cols per chunk
CCH = ECH // DC                 # 128 checks per chunk
N_VCH = 16                      # var chunks
VCH = HV // N_VCH               # 256 vars per chunk
T_LOCA, T_FOR, T_LOCB, T_X = 0, HE, 2 * HE, 3 * HE
T_COLS = 3 * HE + N_VAR         # 45056
WIN = 2 * HE                    # 24576-col sib gather window

EPS = 1e-12
_C = np.float32(1.0) - np.float32(1e-7)
TCLIP = float(np.float32((np.float32(1.0) - _C) / (np.float32(1.0) + _C)))

_CACHE = {}
_LAST_RESULTS = None


def _wrap(stream):
    """Pack an unwrapped per-core index stream [8, n] -> wrapped [128, n//16].

    ap_gather unwraps core k's indices as unwrapped[s*16+p] = tile[16k+p, s].
    """
    st = np.asarray(stream, np.int16)
    ncore, n = st.shape
    assert n % 16 == 0
    out = np.zeros((16 * ncore, n // 16), np.int16)
    for k in range(ncore):
        out[16 * k:16 * (k + 1), :] = st[k].reshape(n // 16, 16).T
    return out


def _build_indices(vi, ci):
    """Host-side graph preprocessing. Returns dict of wrapped index planes."""
    order = np.argsort(ci, kind="stable")          # check-major edge list
    cm_var = vi[order].astype(np.int64)            # var of each cm edge
    pos_of_edge = np.empty(E, np.int64)
    pos_of_edge[order] = np.arange(E)
    edges_of_var = np.argsort(vi, kind="stable").reshape(N_VAR, DV)
    pos_var = pos_of_edge[edges_of_var]            # [N_VAR, 3] cm positions

    half_of_pos = pos_var // HE                    # [N_VAR, 3]

    def rel(p, H, parity):
        # relative coord of global cm position p within the sib window of
        # `parity` (0 = A window [0:24576), 1 = B window [12288:36864)),
        # as seen from a partition in half H.
        same = (p // HE) == H
        if parity == 0:
            return (p % HE) + HE * (~same)
        return (p % HE) + HE * same

    planes = {}
    # sibling + x index streams, per half
    for parity in (0, 1):
        s1 = np.zeros((2, HE), np.int64)
        s2 = np.zeros((2, HE), np.int64)
        for H in (0, 1):
            jj = np.arange(H * HE, (H + 1) * HE)
            v = cm_var[jj]                          # [HE]
            pv = pos_var[v]                         # [HE, 3]
            # sibling positions: the 2 of pv != jj, kept in slot order
            mask = pv != jj[:, None]
            sib = pv[mask].reshape(HE, 2)
            s1[H] = rel(sib[:, 0], H, parity)
            s2[H] = rel(sib[:, 1], H, parity)
        planes[f"s1{'ab'[parity]}"] = _wrap(
            np.concatenate([np.repeat(s1[0][None], 4, 0),
                            np.repeat(s1[1][None], 4, 0)]))
        planes[f"s2{'ab'[parity]}"] = _wrap(
            np.concatenate([np.repeat(s2[0][None], 4, 0),
                            np.repeat(s2[1][None], 4, 0)]))
    xi = np.zeros((2, HE), np.int64)
    for H in (0, 1):
        xi[H] = cm_var[np.arange(H * HE, (H + 1) * HE)]
    planes["xi"] = _wrap(np.concatenate([np.repeat(xi[0][None], 4, 0),
                                         np.repeat(xi[1][None], 4, 0)]))

    # out gathers: var v (local to half H) -> its 3 edge positions
    for parity in (0, 1):
        for s in range(DV):
            vg = np.zeros((2, HV), np.int64)
            for H in (0, 1):
                vids = np.arange(H * HV, (H + 1) * HV)
                vg[H] = rel(pos_var[vids, s], H, parity)
            planes[f"vg{s}{'ab'[parity]}"] = _wrap(
                np.concatenate([np.repeat(vg[0][None], 4, 0),
                                np.repeat(vg[1][None], 4, 0)]))
    return planes


def _build_bass():
    import concourse.bass as bass
    import concourse.tile as tile
    from concourse import bacc, mybir
    from contextlib import ExitStack

    dt = mybir.dt
    F32, I16 = dt.float32, dt.int16
    ALU = mybir.AluOpType
    ACT = mybir.ActivationFunctionType
    AX = mybir.AxisListType

    nc = bacc.Bacc("TRN2", target_bir_lowering=False, debug=False)

    llr_d = nc.dram_tensor("llr", [BC, N_VAR], F32, kind="ExternalInput").ap()
    idx_d = {}
    for nm in ("s1a", "s1b", "s2a", "s2b", "xi"):
        idx_d[nm] = nc.dram_tensor(nm, [128, HE // 16], I16,
                                   kind="ExternalInput").ap()
    for parity in (0, 1):
        for s in range(DV):
            nm = f"vg{s}{'ab'[parity]}"
            idx_d[nm] = nc.dram_tensor(nm, [128, HV // 16], I16,
                                       kind="ExternalInput").ap()
    out_d = nc.dram_tensor("out", [N_ITER, BC, N_VAR], F32,
                           kind="ExternalOutput").ap()

    with tile.TileContext(nc) as tc, ExitStack() as ctx:
        big = ctx.enter_context(tc.tile_pool(name="big", bufs=1))
        wp = ctx.enter_context(tc.tile_pool(name="wp", bufs=1))
        pp = ctx.enter_context(tc.tile_pool(name="pp", bufs=1, space="PSUM"))

        table = big.tile([128, T_COLS], F32, tag="table")
        # persistent smalls: csum|cp1|cp|p3|eps packed in one 4KB tile
        sm = big.tile([128, 772], F32, tag="smalls")
        sm_csum = sm[:, 0:CCH]
        sm_cp1 = sm[:, CCH:2 * CCH]
        sm_cp = sm[:, 2 * CCH:3 * CCH]
        sm_p3 = sm[:, 3 * CCH:6 * CCH]
        sm_eps = sm[:, 768:769]
        nc.vector.memset(sm_eps, EPS)

        # load x region (duplicated across halves)
        nc.sync.dma_start(table[0:64, T_X:T_X + N_VAR], llr_d[:, :])
        nc.sync.dma_start(table[64:128, T_X:T_X + N_VAR], llr_d[:, :])

        xwin = table[:, T_X:T_X + N_VAR]
        IC = ECH // 16          # wrapped idx cols per check chunk (48)
        IVC = VCH // 16         # wrapped idx cols per var chunk (16)

        for it in range(N_ITER):
            side = it % 2
            wr = T_LOCA if side == 0 else T_LOCB
            wloc = table[:, wr:wr + HE]

            # per-iteration index tile: s1|s2|xi|vg0|vg1|vg2
            ixt = wp.tile([128, 3072], I16, tag="idx")
            pab = "ab"[(it - 1) % 2]
            cab = "ab"[side]
            if it > 0:
                nc.sync.dma_start(ixt[:, 0:768], idx_d[f"s1{pab}"][:])
                nc.sync.dma_start(ixt[:, 768:1536], idx_d[f"s2{pab}"][:])
            nc.sync.dma_start(ixt[:, 1536:2304], idx_d["xi"][:])
            for s in range(DV):
                nc.sync.dma_start(ixt[:, 2304 + 256 * s:2304 + 256 * (s + 1)],
                                  idx_d[f"vg{s}{cab}"][:])
            ix_s1 = ixt[:, 0:768]
            ix_s2 = ixt[:, 768:1536]
            ix_xi = ixt[:, 1536:2304]

            if it > 0:
                pwin_off = T_LOCA if (it - 1) % 2 == 0 else T_FOR
                pwin = table[:, pwin_off:pwin_off + WIN]

            for c in range(N_ECH):
                cl = slice(c * ECH, (c + 1) * ECH)
                ic = slice(c * IC, (c + 1) * IC)
                G = wp.tile([128, 3 * ECH], F32, tag="G")
                g1, g2, g3 = G[:, 0:ECH], G[:, ECH:2 * ECH], G[:, 2 * ECH:3 * ECH]
                nc.gpsimd.ap_gather(g3, xwin, ix_xi[:, ic],
                                    channels=128, num_elems=N_VAR, d=1,
                                    num_idxs=ECH)
                if it == 0:
                    msg_ap = g3
                else:
                    nc.gpsimd.ap_gather(g1, pwin, ix_s1[:, ic],
                                        channels=128, num_elems=WIN, d=1,
                                        num_idxs=ECH)
                    nc.gpsimd.ap_gather(g2, pwin, ix_s2[:, ic],
                                        channels=128, num_elems=WIN, d=1,
                                        num_idxs=ECH)
                    pm = pp.tile([128, ECH], F32, tag="P1")
                    nc.vector.tensor_tensor(pm[:], g1, g2, op=ALU.add)
                    msg = pp.tile([128, ECH], F32, tag="P2")
                    nc.vector.tensor_tensor(msg[:], pm[:], g3, op=ALU.add)
                    msg_ap = msg[:]

                tt = pp.tile([128, ECH], F32, tag="P1")
                nc.scalar.activation(tt[:], msg_ap, ACT.Tanh, scale=0.5)
                sg = wp.tile([128, ECH], F32, tag="sgx")
                nc.scalar.activation(sg[:], tt[:], ACT.Sign)
                ab = pp.tile([128, ECH], F32, tag="P2")
                nc.scalar.activation(ab[:], tt[:], ACT.Abs)
                la = pp.tile([128, ECH], F32, tag="P3")
                nc.scalar.activation(la[:], ab[:], ACT.Ln, bias=sm_eps)

                la6 = la[:].rearrange("p (n k) -> p n k", k=DC)
                nc.vector.tensor_reduce(sm_csum, la6, axis=AX.X, op=ALU.add)

                sg6 = sg[:].rearrange("p (n k) -> p n k", k=DC)
                p3v = sm_p3.rearrange("p (n k) -> p n k", k=3)
                nc.vector.tensor_tensor(p3v, sg6[:, :, 0:3], sg6[:, :, 3:6],
                                        op=ALU.mult)
                nc.vector.tensor_tensor(sm_cp1, p3v[:, :, 0], p3v[:, :, 1],
                                        op=ALU.mult)
                nc.vector.tensor_tensor(sm_cp, sm_cp1, p3v[:, :, 2],
                                        op=ALU.mult)

                dd = pp.tile([128, ECH], F32, tag="P4")
                dd6 = dd[:].rearrange("p (n k) -> p n k", k=DC)
                csb = sm_csum.unsqueeze(2).broadcast_to([128, CCH, DC])
                nc.vector.tensor_tensor(dd6, csb, la6, op=ALU.subtract)

                t2 = pp.tile([128, ECH], F32, tag="P1")
                nc.scalar.activation(t2[:], dd[:], ACT.Tanh, scale=-0.5)
                t2c = pp.tile([128, ECH], F32, tag="P2")
                nc.vector.tensor_scalar_max(t2c[:], t2[:], TCLIP)

                se = pp.tile([128, ECH], F32, tag="P4")
                se6 = se[:].rearrange("p (n k) -> p n k", k=DC)
                cpb = sm_cp.unsqueeze(2).broadcast_to([128, CCH, DC])
                nc.vector.tensor_tensor(se6, sg6, cpb, op=ALU.mult)

                a5 = wp.tile([128, ECH], F32, tag="sgx")
                nc.scalar.activation(a5[:], t2c[:], ACT.Ln)

                # ext = (-a5) * se  ->  local write side
                nc.vector.scalar_tensor_tensor(
                    wloc[:, cl], a5[:], -1.0, se[:],
                    op0=ALU.mult, op1=ALU.mult)

            # cross-fill to the partner half's foreign region. Emitted after
            # ALL of this iteration's sibling gathers so they still read the
            # previous state's foreign values (the region is single-buffered).
            for c in range(N_ECH):
                cl = slice(c * ECH, (c + 1) * ECH)
                nc.sync.dma_start(
                    table[0:64, T_FOR:T_FOR + HE][:, cl],
                    wloc[64:128, cl])
                nc.sync.dma_start(
                    table[64:128, T_FOR:T_FOR + HE][:, cl],
                    wloc[0:64, cl])

            # out phase: reads current state window
            cwin_off = T_LOCA if side == 0 else T_FOR
            cwin = table[:, cwin_off:cwin_off + WIN]
            for vc in range(N_VCH):
                iv = slice(vc * IVC, (vc + 1) * IVC)
                geo = wp.tile([128, 3 * VCH], F32, tag="G")
                ge = [geo[:, s * VCH:(s + 1) * VCH] for s in range(DV)]
                for s in range(DV):
                    nc.gpsimd.ap_gather(
                        ge[s], cwin, ixt[:, 2304 + 256 * s:2304 + 256 * (s + 1)][:, iv],
                        channels=128, num_elems=WIN, d=1, num_idxs=VCH)
                vso = wp.tile([128, 2 * VCH], F32, tag="sgx")
                vs, ov = vso[:, 0:VCH], vso[:, VCH:2 * VCH]
                nc.vector.tensor_tensor(vs, ge[0], ge[1], op=ALU.add)
                nc.vector.tensor_tensor(ov, vs, ge[2], op=ALU.add)
                vl = slice(vc * VCH, (vc + 1) * VCH)
                nc.vector.tensor_tensor(
                    vs[0:64], ov[0:64],
                    table[0:64, T_X:T_X + HV][:, vl], op=ALU.add)
                nc.vector.tensor_tensor(
                    vs[64:128], ov[64:128],
                    table[64:128, T_X + HV:T_X + N_VAR][:, vl], op=ALU.add)
                nc.sync.dma_start(out_d[it, :, vc * VCH:(vc + 1) * VCH],
                                  vs[0:64])
                nc.sync.dma_start(
                    out_d[it, :, HV + vc * VCH:HV + (vc + 1) * VCH],
                    vs[64:128])

    nc.compile()
    return nc


def _numpy_fallback(llr, vi, ci):
    x = llr.T.astype(np.float32)
    scattered = x[vi]
    ext = np.zeros_like(scattered)
    outs = []
    for _ in range(N_ITER):
        vsum = np.zeros((N_VAR, x.shape[1]), np.float32)
        np.add.at(vsum, vi, ext)
        msg = (vsum[vi] - ext) + scattered
        t = np.tanh(msg * 0.5)
        la = np.log(np.abs(t) + EPS)
        sg = np.sign(t)
        cs = np.zeros((N_CHK, x.shape[1]), np.float32)
        np.add.at(cs, ci, la)
        cpr = np.ones((N_CHK, x.shape[1]), np.float32)
        np.multiply.at(cpr, ci, sg)
        loo = np.exp(cs[ci] - la) * (cpr[ci] * sg)
        loo = np.clip(loo, -float(_C), float(_C))
        ext = 2.0 * np.arctanh(loo)
        vs2 = np.zeros((N_VAR, x.shape[1]), np.float32)
        np.add.at(vs2, vi, ext)
        outs.append((vs2 + x).T)
    return np.stack(outs)


def kernel(llr, var_index, chk_index):
    llr = np.asarray(llr, np.float32)
    vi = np.asarray(var_index, np.int64).ravel()
    ci = np.asarray(chk_index, np.int64).ravel()
    assert llr.shape == (BATCH, N_VAR) and vi.shape == (E,) and ci.shape == (E,)

    regular = (np.array_equal(np.bincount(vi, minlength=N_VAR),
                              np.full(N_VAR, DV))
               and np.array_equal(np.bincount(ci, minlength=N_CHK),
                                  np.full(N_CHK, DC)))
    if not regular:
        return _numpy_fallback(llr, vi, ci).astype(np.float32)

    key = ("k", hash(vi.tobytes()), hash(ci.tobytes()))
    if key not in _CACHE:
        planes = _build_indices(vi, ci)
        nc = _build_bass()
        _CACHE[key] = (nc, planes)
    nc, planes = _CACHE[key]

    from concourse.bass_utils import run_bass_kernel_spmd
    in_maps = []
    for c in range(N_CORES):
        m = {nm: np.ascontiguousarray(v) for nm, v in planes.items()}
        m["llr"] = np.ascontiguousarray(llr[c * BC:(c + 1) * BC, :])
        in_maps.append(m)
    trace = os.environ.get("BASS_KERNEL_TRACE", "0") == "1"
    res = run_bass_kernel_spmd(nc, in_maps, list(range(N_CORES)), trace=trace)
    global _LAST_RESULTS
    _LAST_RESULTS = res
    out = np.concatenate([res.results[c]["out"] for c in range(N_CORES)],
                         axis=1)
    return np.ascontiguousarray(out, dtype=np.float32)


if __name__ == "__main__":
    sys.path.insert(0, os.path.dirname(os.path.abspath(__file__)))
    import reference
    inputs = {k: np.asarray(v) for k, v in reference.setup_inputs().items()}
    exp = np.asarray(reference.reference(**inputs))
    got = kernel(**inputs)
    err = np.max(np.abs(got - exp)) / (np.max(np.abs(exp)) + 1e-30)
    print("Relative error:", err)



# revision 2
# speedup vs baseline: 1.0441x; 1.0441x over previous
"""Trainium2 Bass kernel v2 for NeuralSumProductModel (LDPC sum-product).

Design (per NeuronCore, batch sharded 512 -> 8 x 64, TRANSPOSED layout):
  All edge/var arrays live as [128 partitions, slots, 64 batch] with batch
  contiguous innermost (one edge/var row = 64 f32 = 256B).

  check-major (cm): partition p owns checks [32p, 32p+32); slot j in [0,192)
    = (check-local j//6, edge t=j%6).  Check reductions are strided DVE ops.
  var-major (vm): partition p owns vars [64p, 64p+64); slot j = (v-local
    j//3, s=j%3).  Var sums are strided DVE ops.

  The graph permutations (vm<->cm) are done with gpsimd.dma_gather through
  HBM staging buffers: SBUF -> HBM is a plain strided DMA (contiguous 256B
  rows), HBM -> SBUF is a row-gather (SWDGE descriptors, ~0.34ns/desc,
  spread over 16 DMA engines).  No GPSIMD ap_gather (which costs ~30-100
  Q7 cycles per index) anywhere.

  Per iteration:
    C phase (cm): msg -> t=tanh(.5 msg), sg=sign, la=ln|t|+eps, csum/cprod
      over each check's 6 edges, d=csum-la, ext=-ln(max(tanh(-d/2),TCLIP))
      * sg*cprod.  Activations phase-batched so the ACT engine only swaps
      function tables 4x per iteration (tanh and ln live in different sets).
    w1: ext_cm -> stage1 (strided DMA);  gv: gather stage1 -> ext_vm.
    A phase (vm): out = e0+e1+e2+x -> out5[it] (HBM);  msg_vm = out - ext.
    w2: msg_vm -> stage2;  gc: gather stage2 -> msg_cm (next iteration).
  Iteration 0 gathers x[var(e)] rows directly from the transposed-LLR input.

  fp16 is used for the bounded intermediate tensors (t, sign, ln|t|, csum,
  d) for SBUF capacity + DVE 2x mode; the clip chain (tanh(-d/2) in
  [TCLIP, 1], TCLIP~5e-8) must stay fp32.

  Host side: llr is transposed (and fp16-cast) before upload; the output
  [5, 8192, 64] per core is transposed back — host work is not on the
  device critical path.
"""

import os
import sys

import numpy as np

for _p in ("/opt/trn_rl_repo", "/root/.axon_site/_ro/trn_rl_repo"):
    if os.path.isdir(_p) and _p not in sys.path:
        sys.path.insert(0, _p)

N_VAR, N_CHK, DV, DC = 8192, 4096, 3, 6
E = N_VAR * DV  # 24576
BATCH, N_ITER, N_CORES = 512, int(os.environ.get("K2_ITERS", "5")), 8
BC = BATCH // N_CORES        # 64 batch rows per core
SL = E // 128                # 192 slots per partition
HSL = SL // 2                # 96 slots per compute half
DSL = SL // 4                # 48 slots per DMA chunk
TOK = 128 * DSL              # 6144 tokens per gather chunk
CHK_H = HSL // DC            # 16 checks per half per partition
VH = 32                      # vars per half per partition (64/2)

EPS = 1e-12
_C = np.float32(1.0) - np.float32(1e-7)
TCLIP = float(np.float32((np.float32(1.0) - _C) / (np.float32(1.0) + _C)))

_CACHE = {}
_LAST_RESULTS = None


def _wrap_tokens(u):
    """u: [E] token-order values -> wrapped idx plane [128, E//16] i16.

    Gathers are issued in 4 chunks of TOK tokens; each chunk's indices are
    wrapped independently: within chunk m, token i' sits at
    [p, 384*m + i'//16] for p%16 == i'%16 (replicated across the 8 cores).
    """
    u = np.asarray(u, np.int64)
    out = np.zeros((128, E // 16), np.int16)
    for m in range(4):
        c = u[m * TOK:(m + 1) * TOK].reshape(-1, 16).T  # [16, 384]
        for g in range(8):
            out[16 * g:16 * (g + 1), 384 * m:384 * (m + 1)] = c
    return out


def _build_indices(vi, ci):
    """Host-side graph preprocessing -> three wrapped index planes.

    Token i of a gather lands at SBUF (p=i%128, slot=i//128) within its
    chunk; chunk m covers slots [48m, 48m+48).
    stage rows:  cm row r1(p,j) = 192p + j;  vm row r2(v,s) = 192*(v//64)
    + 3*(v%64) + s.
    """
    order = np.argsort(ci, kind="stable")          # cm position q -> edge id
    pos_of_edge = np.empty(E, np.int64)
    pos_of_edge[order] = np.arange(E)
    edges_of_var = np.argsort(vi, kind="stable").reshape(N_VAR, DV)
    cm_var = vi[order].astype(np.int64)            # var of cm position q

    # token i -> (p, slot j) -> global position q = 192p + j
    i = np.arange(E)
    p = i % 128
    j = i // 128
    jc = j % DSL
    m = j // DSL
    jj = m * DSL + jc                              # == j
    q = 192 * p + jj

    u_x = cm_var[q]                                # x-row gather (iter 0)

    # vm slot of each edge
    v_of = cm_var                                  # var of cm position
    slot_in_var = np.zeros(E, np.int64)
    for s in range(DV):
        slot_in_var[pos_of_edge[edges_of_var[:, s]]] = s
    r2_of_q = 192 * (v_of // 64) + 3 * (v_of % 64) + slot_in_var
    u_v2c = r2_of_q[q]                             # msg_cm <- stage2(vm rows)

    # vm token i -> (p, j): v = 64p + j//3, s = j%3 -> cm row of that edge
    v = 64 * p + j // DV
    s = j % DV
    u_c2v = pos_of_edge[edges_of_var[v, s]]        # ext_vm <- stage1(cm rows)

    return {
        "ix": _wrap_tokens(u_x),
        "icv": _wrap_tokens(u_c2v),
        "ivc": _wrap_tokens(u_v2c),
    }


def _build_bass():
    import concourse.bass as bass  # noqa: F401  (side-effect imports)
    from concourse import bacc, mybir

    STOP = int(os.environ.get("K2_STOP", "10"))
    DBG = os.environ.get("K2_DEBUG", "0") == "1"
    DBGIT = int(os.environ.get("K2_DBGIT", "0"))

    dt = mybir.dt
    F32, F16, I16 = dt.float32, dt.float16, dt.int16
    ALU = mybir.AluOpType
    ACT = mybir.ActivationFunctionType

    nc = bacc.Bacc("TRN2", target_bir_lowering=False, debug=False)

    xT_d = nc.dram_tensor("xT", [N_VAR, BC], F32, kind="ExternalInput").ap()
    x16_d = nc.dram_tensor("x16", [N_VAR, BC], F16, kind="ExternalInput").ap()
    ix_d = nc.dram_tensor("ix", [128, E // 16], I16, kind="ExternalInput").ap()
    icv_d = nc.dram_tensor("icv", [128, E // 16], I16,
                           kind="ExternalInput").ap()
    ivc_d = nc.dram_tensor("ivc", [128, E // 16], I16,
                           kind="ExternalInput").ap()
    out_d = nc.dram_tensor("out", [N_ITER, N_VAR, BC], F32,
                           kind="ExternalOutput").ap()
    stg1_d = nc.dram_tensor("stg1", [E, BC], F32).ap()
    dbg = {}
    stg2_d = nc.dram_tensor("stg2", [E, BC], F32).ap()

    bufA = nc.alloc_sbuf_tensor("bufA", [128, SL, BC], F32).ap()
    bufB = nc.alloc_sbuf_tensor("bufB", [128, SL, BC], F32).ap()
    xsb = nc.alloc_sbuf_tensor("xsb", [128, 64, BC], F16).ap()
    ixx = nc.alloc_sbuf_tensor("ixx", [128, E // 16], I16).ap()
    ixcv = nc.alloc_sbuf_tensor("ixcv", [128, E // 16], I16).ap()
    ixvc = nc.alloc_sbuf_tensor("ixvc", [128, E // 16], I16).ap()
    BF16 = dt.bfloat16
    QF = HSL * BC // 2                 # 3072 f32 per quarter per partition
    CHK_Q = CHK_H // 2                 # 8 checks per quarter
    X1 = nc.alloc_sbuf_tensor("X1", [128, 4, QF], BF16).ap()
    X2 = nc.alloc_sbuf_tensor("X2", [128, 4, QF], BF16).ap()
    W = nc.alloc_sbuf_tensor("W", [128, 2, QF], F32).ap()
    CS = nc.alloc_sbuf_tensor("CS", [128, CHK_Q, BC], F32).ap()
    TS1 = nc.alloc_sbuf_tensor("TS1", [128, CHK_Q, BC], F32).ap()
    CP = nc.alloc_sbuf_tensor("CP", [128, 4, CHK_Q, BC], BF16).ap()
    TP1 = nc.alloc_sbuf_tensor("TP1", [128, CHK_Q, BC], BF16).ap()
    epsb = nc.alloc_sbuf_tensor("epsb", [128, 1], F32).ap()
    oneb = nc.alloc_sbuf_tensor("oneb", [128, 1], F32).ap()
    onepb = nc.alloc_sbuf_tensor("onepb", [128, 1], F32).ap()
    ov = nc.alloc_sbuf_tensor("ov", [128, VH, BC], F32).ap()

    stg1v = stg1_d.rearrange("(p r) b -> p r b", p=128)
    stg2v = stg2_d.rearrange("(p r) b -> p r b", p=128)

    sems = {}
    marks = {}

    def sem(name):
        if name not in sems:
            sems[name] = nc.alloc_semaphore(name)
        return sems[name]

    def wait(eng, name, val):
        if val <= 0:
            return
        key = (id(eng), name)
        if marks.get(key, -1) >= val:
            return
        eng.wait_ge(sem(name), val)
        marks[key] = val

    ACTE, DVE, GPS, SP = nc.scalar, nc.vector, nc.gpsimd, nc.sync

    # ---- prologue ----
    SP.dma_start(ixx, ix_d).then_inc(sem("ld"), 16)
    SP.dma_start(ixcv, icv_d).then_inc(sem("ld"), 16)
    SP.dma_start(ixvc, ivc_d).then_inc(sem("ld"), 16)
    SP.dma_start(xsb, x16_d.rearrange("(p v) b -> p v b", p=128)
                 ).then_inc(sem("ld"), 16)
    DVE.memset(epsb, EPS)
    DVE.memset(oneb, 1.0)
    DVE.memset(onepb, 1.0 + 1e-7).then_inc(sem("epsv"), 1)

    wait(GPS, "ld", 64)
    for m in range(4):
        GPS.dma_gather(bufA[:, DSL * m:DSL * (m + 1), :], xT_d,
                       ixx[:, 384 * m:384 * (m + 1)], TOK, TOK, BC,
                       single_packet=False).then_inc(sem(f"gc{m}"), 16)

    def half_views(buf, h):
        sl = buf[:, HSL * h:HSL * (h + 1), :]
        return (sl.rearrange("p r b -> p (r b)"),
                sl.rearrange("p (c t) b -> p c t b", t=DC))

    for it in range(N_ITER):
        gcb = 64 * it
        # ---- C phase: 4 quarters; PASS-A all in natural_log_exp set,
        # PASS-B tanh (set E), PASS-C ln (set L): 2 table switches/iter.
        # la = ln(tanh(|m|/2)+~eps) = ln(1-y+eps)-ln(1+y), y=e^-|m| (the
        # 1e-5-rel Exp table only feeds la, where it is harmless); the
        # magnitude tail runs through the 6e-8-rel Tanh table:
        # ext = -ln(max(tanh(-d/2), TCLIP)) * sg * cprod.
        # bf16 stores la/d/t2c/sg (floating exponent: no subnormal flush);
        # csum stays f32 so d = csum - la_e cancels la_e's stored bits.
        for q in range(4):
            n = 4 * it + q + 1
            mvq = bufA[:, DSL * q:DSL * (q + 1), :].rearrange(
                "p r b -> p (r b)")
            w = W[:, q % 2, :]
            x1 = X1[:, q, :]
            x2 = X2[:, q, :]
            la = x1.rearrange("p (c t b) -> p c t b", t=DC, b=BC)
            sg = x2.rearrange("p (c t b) -> p c t b", t=DC, b=BC)
            # -- PASS-A (set L): am, y, l1, l2, la, sg, trees, d, q-prod --
            if DBG and it == DBGIT:
                if "msg" not in dbg:
                    dbg["msg"] = nc.dram_tensor(
                        "dbg_msg", [4, 128, QF], F32,
                        kind="ExternalOutput").ap()
                wait(SP, f"gc{q}", 16 * (it + 1))
                SP.dma_start(dbg["msg"][q], mvq).then_inc(sem("dbgM"), 16)
            wait(ACTE, f"gc{q}", 16 * (it + 1))
            wait(ACTE, "ext", 4 * it)      # X2 WAR vs prev iter ext
            if q >= 2:
                wait(ACTE, "dla", 4 * it + q - 1)  # W WAR vs q-2 la read
            ACTE.activation(w, mvq, ACT.Abs).then_inc(sem("aam"), 1)
            wait(ACTE, "aam", n)
            ACTE.activation(w, w, ACT.Exp, scale=-1.0).then_inc(sem("ay"), 1)
            wait(ACTE, "ay", n)
            wait(ACTE, "epsv", 1)
            ACTE.activation(x2, w, ACT.Ln, scale=-1.0,
                            bias=onepb).then_inc(sem("al1"), 1)
            wait(ACTE, "al1", n)
            ACTE.activation(w, w, ACT.Ln, bias=oneb).then_inc(sem("al2"), 1)
            wait(DVE, "al2", n)
            DVE.tensor_tensor(x1, x2, w,
                              op=ALU.subtract).then_inc(sem("dla"), 1)
            if DBG and it == DBGIT:
                if "la" not in dbg:
                    dbg["la"] = nc.dram_tensor(
                        "dbg_la", [4, 128, QF], dt.bfloat16,
                        kind="ExternalOutput").ap()
                wait(SP, "dla", n)
                SP.dma_start(dbg["la"][q], x1).then_inc(sem("dbgA"), 16)
            wait(ACTE, "dla", n)
            ACTE.activation(x2, mvq, ACT.Sign).then_inc(sem("asg"), 1)
            rt = 10 * (4 * it + q)
            cp = CP[:, q, :, :]
            wait(DVE, "asg", n)
            wait(DVE, "dved", n - 1)       # CS/TS1 WAR vs prev quarter d
            DVE.tensor_tensor(CS, la[:, :, 0, :], la[:, :, 1, :],
                              op=ALU.add).then_inc(sem("rt"), 1)
            DVE.tensor_tensor(TS1, la[:, :, 2, :], la[:, :, 3, :],
                              op=ALU.add).then_inc(sem("rt"), 1)
            DVE.tensor_tensor(cp, sg[:, :, 0, :], sg[:, :, 1, :],
                              op=ALU.mult).then_inc(sem("rt"), 1)
            DVE.tensor_tensor(TP1, sg[:, :, 2, :], sg[:, :, 3, :],
                              op=ALU.mult).then_inc(sem("rt"), 1)
            wait(DVE, "rt", rt + 4)
            DVE.tensor_tensor(CS, CS, TS1, op=ALU.add).then_inc(sem("rt"), 1)
            DVE.tensor_tensor(cp, cp, TP1,
                              op=ALU.mult).then_inc(sem("rt"), 1)
            wait(DVE, "rt", rt + 6)
            DVE.tensor_tensor(TS1, la[:, :, 4, :], la[:, :, 5, :],
                              op=ALU.add).then_inc(sem("rt"), 1)
            DVE.tensor_tensor(TP1, sg[:, :, 4, :], sg[:, :, 5, :],
                              op=ALU.mult).then_inc(sem("rt"), 1)
            wait(DVE, "rt", rt + 8)
            DVE.tensor_tensor(CS, CS, TS1, op=ALU.add).then_inc(sem("rt"), 1)
            DVE.tensor_tensor(cp, cp, TP1,
                              op=ALU.mult).then_inc(sem("rt"), 1)
            wait(DVE, "rt", rt + 10)
            if DBG and it == DBGIT:
                wait(DVE, "dbgA", 16 * (q + 1))
            csb = CS.unsqueeze(2).broadcast_to([128, CHK_Q, DC, BC])
            DVE.tensor_tensor(la, csb, la,
                              op=ALU.subtract).then_inc(sem("dved"), 1)
            cpb = cp.unsqueeze(2).broadcast_to([128, CHK_Q, DC, BC])
            DVE.tensor_tensor(sg, sg, cpb,
                              op=ALU.mult).then_inc(sem("dveq"), 1)
            if DBG and it == DBGIT:
                if "d" not in dbg:
                    dbg["d"] = nc.dram_tensor(
                        "dbg_d", [4, 128, QF], dt.bfloat16,
                        kind="ExternalOutput").ap()
                    dbg["q"] = nc.dram_tensor(
                        "dbg_q", [4, 128, QF], dt.bfloat16,
                        kind="ExternalOutput").ap()
                wait(SP, "dved", n)
                SP.dma_start(dbg["d"][q], x1).then_inc(sem("dbgB"), 16)
                wait(SP, "dveq", n)
                SP.dma_start(dbg["q"][q], x2).then_inc(sem("dbgE"), 16)
        # -- PASS-B (set E): t2 = tanh(-d/2), clip on DVE (bf16 2x) --
        for q in range(4):
            x1 = X1[:, q, :]
            wait(ACTE, "dved", 4 * it + q + 1)
            if DBG and it == DBGIT:
                wait(ACTE, "dbgB", 16 * (q + 1))
            ACTE.activation(x1, x1, ACT.Tanh,
                            scale=-0.5).then_inc(sem("actu"), 1)
            wait(DVE, "actu", 4 * it + q + 1)
            DVE.tensor_scalar_max(x1, x1, TCLIP).then_inc(sem("gmx"), 1)
        if DBG and it == DBGIT:
            if "t2c" not in dbg:
                dbg["t2c"] = nc.dram_tensor(
                    "dbg_t2c", [128, 4, QF], dt.bfloat16,
                    kind="ExternalOutput").ap()
            wait(SP, "gmx", 4 * it + 4)
            SP.dma_start(dbg["t2c"],
                         X1.rearrange("p q f -> p (q f)")
                         ).then_inc(sem("dbgC"), 16)
        # -- PASS-C (set L): V = ln(t2c) in f32 (W); ext = (-V)*sg*cprod --
        for q in range(4):
            x1 = X1[:, q, :]
            w = W[:, q % 2, :]
            wait(ACTE, "gmx", 4 * it + q + 1)
            if DBG and it == DBGIT:
                wait(ACTE, "dbgC", 16)
            if q >= 2:
                wait(ACTE, "ext", 4 * it + q - 1)  # W WAR vs ext q-2
            ACTE.activation(w, x1, ACT.Ln).then_inc(sem("actv"), 1)
            extq = bufB[:, DSL * q:DSL * (q + 1), :].rearrange(
                "p r b -> p (r b)")
            wait(DVE, "actv", 4 * it + q + 1)
            wait(DVE, "dveq", 4 * it + q + 1)
            DVE.scalar_tensor_tensor(extq, w, -1.0, X2[:, q, :],
                                     op0=ALU.mult,
                                     op1=ALU.mult).then_inc(sem("ext"), 1)
        if STOP < 7:
            break
        # ---- ext_cm -> stage1 ----
        for m in range(4):
            wait(SP, "ext", 4 * it + m + 1)
            SP.dma_start(stg1v[:, DSL * m:DSL * (m + 1), :],
                         bufB[:, DSL * m:DSL * (m + 1), :]
                         ).then_inc(sem("w1"), 16)
        if STOP < 8:
            break
        wait(GPS, "w1", 64 * (it + 1))
        for m in range(4):
            GPS.dma_gather(bufA[:, DSL * m:DSL * (m + 1), :], stg1_d,
                           ixcv[:, 384 * m:384 * (m + 1)], TOK, TOK, BC,
                           single_packet=False).then_inc(sem(f"gv{m}"), 16)
        if STOP < 9:
            break
        # ---- A phase (vm): out rows + next msg ----
        for jh in range(2):
            ev = bufA[:, HSL * jh:HSL * (jh + 1), :].rearrange(
                "p (v s) b -> p v s b", s=DV)
            ovj = ov
            wait(DVE, f"gv{2 * jh}", 16 * (it + 1))
            wait(DVE, f"gv{2 * jh + 1}", 16 * (it + 1))
            if jh == 0:
                wait(DVE, "ow1", 16 * it)      # ov WAR vs prev iter half 1
                wait(DVE, "msg", 2 * it)       # ov WAR vs prev iter msg
            else:
                wait(DVE, "ow0", 16 * (it + 1))  # ov WAR vs half 0 this iter
                if it < N_ITER - 1:
                    wait(DVE, "msg", 2 * it + 1)  # ov WAR vs msg half 0
            DVE.tensor_tensor(ovj, ev[:, :, 0, :], ev[:, :, 1, :],
                              op=ALU.add).then_inc(sem("av1"), 1)
            wait(DVE, "av1", 2 * it + jh + 1)
            DVE.tensor_tensor(ovj, ovj, ev[:, :, 2, :],
                              op=ALU.add).then_inc(sem("av2"), 1)
            wait(DVE, "av2", 2 * it + jh + 1)
            DVE.tensor_tensor(ovj, ovj, xsb[:, VH * jh:VH * (jh + 1), :],
                              op=ALU.add).then_inc(sem("dvo"), 1)
            wait(SP, "dvo", 2 * it + jh + 1)
            SP.dma_start(
                out_d[it].rearrange("(p v) b -> p v b", p=128)
                [:, VH * jh:VH * (jh + 1), :],
                ovj).then_inc(sem(f"ow{jh}"), 16)
            if it < N_ITER - 1:
                wait(DVE, "dvo", 2 * it + jh + 1)
                ovb = ovj.unsqueeze(2).broadcast_to([128, VH, DV, BC])
                mgv = bufB[:, HSL * jh:HSL * (jh + 1), :].rearrange(
                    "p (v s) b -> p v s b", s=DV)
                DVE.tensor_tensor(mgv, ovb, ev,
                                  op=ALU.subtract).then_inc(sem("msg"), 1)
        if STOP < 10:
            break
        # ---- msg_vm -> stage2, gather -> msg_cm ----
        if it < N_ITER - 1:
            for m in range(2):
                wait(SP, "msg", 2 * it + m + 1)
                SP.dma_start(stg2v[:, HSL * m:HSL * (m + 1), :],
                             bufB[:, HSL * m:HSL * (m + 1), :]
                             ).then_inc(sem("w2"), 16)
            wait(GPS, "w2", 32 * (it + 1))
            for m in range(4):
                GPS.dma_gather(bufA[:, DSL * m:DSL * (m + 1), :], stg2_d,
                               ixvc[:, 384 * m:384 * (m + 1)], TOK, TOK, BC,
                               single_packet=False).then_inc(sem(f"gc{m}"), 16)

    ni = N_ITER if STOP >= 10 else 1
    if STOP >= 9:
        wait(SP, "ow0", 16 * ni)
        wait(SP, "ow1", 16 * ni)
    elif STOP >= 8:
        wait(SP, "gv0", 32 * ni)
        wait(SP, "gv1", 32 * ni)
    elif STOP >= 7:
        wait(SP, "w1", 32 * ni)
    elif STOP >= 6:
        wait(SP, "ext", 2 * ni)
    elif STOP >= 5:
        wait(SP, "dvem", 2 * ni)
    elif STOP >= 4:
        wait(SP, "dved", 2 * ni)
    elif STOP >= 3:
        wait(SP, "dla", 2 * ni)
    elif STOP >= 2:
        wait(SP, "al2", 2 * ni)
    else:
        for m in range(4):
            wait(SP, f"gc{m}", 16)

    nc.compile()
    return nc


def _numpy_fallback(llr, vi, ci):
    x = llr.T.astype(np.float32)
    scattered = x[vi]
    ext = np.zeros_like(scattered)
    outs = []
    for _ in range(N_ITER):
        vsum = np.zeros((N_VAR, x.shape[1]), np.float32)
        np.add.at(vsum, vi, ext)
        msg = (vsum[vi] - ext) + scattered
        t = np.tanh(msg * 0.5)
        la = np.log(np.abs(t) + EPS)
        sg = np.sign(t)
        cs = np.zeros((N_CHK, x.shape[1]), np.float32)
        np.add.at(cs, ci, la)
        cpr = np.ones((N_CHK, x.shape[1]), np.float32)
        np.multiply.at(cpr, ci, sg)
        loo = np.exp(cs[ci] - la) * (cpr[ci] * sg)
        loo = np.clip(loo, -float(_C), float(_C))
        ext = 2.0 * np.arctanh(loo)
        vs2 = np.zeros((N_VAR, x.shape[1]), np.float32)
        np.add.at(vs2, vi, ext)
        outs.append((vs2 + x).T)
    return np.stack(outs)


def _core_inputs(llr, planes, c):
    xt = np.ascontiguousarray(llr[c * BC:(c + 1) * BC, :].T, np.float32)
    m = {nm: np.ascontiguousarray(v) for nm, v in planes.items()}
    m["xT"] = xt
    m["x16"] = xt.astype(np.float16)
    return m


def kernel(llr, var_index, chk_index):
    llr = np.asarray(llr, np.float32)
    vi = np.asarray(var_index, np.int64).ravel()
    ci = np.asarray(chk_index, np.int64).ravel()
    assert llr.shape == (BATCH, N_VAR) and vi.shape == (E,) and ci.shape == (E,)

    regular = (np.array_equal(np.bincount(vi, minlength=N_VAR),
                              np.full(N_VAR, DV))
               and np.array_equal(np.bincount(ci, minlength=N_CHK),
                                  np.full(N_CHK, DC)))
    if not regular:
        return _numpy_fallback(llr, vi, ci).astype(np.float32)

    key = ("k2", hash(vi.tobytes()), hash(ci.tobytes()))
    if key not in _CACHE:
        planes = _build_indices(vi, ci)
        nc = _build_bass()
        _CACHE[key] = (nc, planes)
    nc, planes = _CACHE[key]

    from concourse.bass_utils import run_bass_kernel_spmd
    in_maps = [_core_inputs(llr, planes, c) for c in range(N_CORES)]
    trace = os.environ.get("BASS_KERNEL_TRACE", "0") == "1"
    res = run_bass_kernel_spmd(nc, in_maps, list(range(N_CORES)), trace=trace)
    global _LAST_RESULTS
    _LAST_RESULTS = res
    out = np.concatenate(
        [np.transpose(res.results[c]["out"], (0, 2, 1))
         for c in range(N_CORES)], axis=1)
    return np.ascontiguousarray(out, dtype=np.float32)


if __name__ == "__main__":
    sys.path.insert(0, os.path.dirname(os.path.abspath(__file__)))
    import reference
    inputs = {k: np.asarray(v) for k, v in reference.setup_inputs().items()}
    llr = np.asarray(inputs["llr"], np.float32)
    vi = np.asarray(inputs["var_index"], np.int64)
    ci = np.asarray(inputs["chk_index"], np.int64)
    exp = _numpy_fallback(llr, vi, ci)

    if "--sim" in sys.argv:
        from concourse.bass_interp import CoreSim
        planes = _build_indices(vi, ci)
        nc = _build_bass()
        sim = CoreSim(nc)
        for k, v in _core_inputs(llr, planes, 0).items():
            sim.tensor(k)[:] = v
        sim.simulate()
        got0 = np.transpose(np.asarray(sim.tensor("out")), (0, 2, 1))
        exp0 = exp[:, 0:BC, :]
        err = np.max(np.abs(got0 - exp0)) / (np.max(np.abs(exp0)) + 1e-30)
        print("SIM relative error (core 0):", err)
    else:
        got = kernel(**inputs)
        err = np.max(np.abs(got - exp)) / (np.max(np.abs(exp)) + 1e-30)
        print("Relative error:", err)


# revision 3
# speedup vs baseline: 1.0604x; 1.0156x over previous
"""Trainium2 Bass kernel v2 for NeuralSumProductModel (LDPC sum-product).

Design (per NeuronCore, batch sharded 512 -> 8 x 64, TRANSPOSED layout):
  All edge/var arrays live as [128 partitions, slots, 64 batch] with batch
  contiguous innermost (one edge/var row = 64 f32 = 256B).

  check-major (cm): partition p owns checks [32p, 32p+32); slot j in [0,192)
    = (check-local j//6, edge t=j%6).  Check reductions are strided DVE ops.
  var-major (vm): partition p owns vars [64p, 64p+64); slot j = (v-local
    j//3, s=j%3).  Var sums are strided DVE ops.

  The graph permutations (vm<->cm) are done with gpsimd.dma_gather through
  HBM staging buffers: SBUF -> HBM is a plain strided DMA (contiguous 256B
  rows), HBM -> SBUF is a row-gather (SWDGE descriptors, ~0.34ns/desc,
  spread over 16 DMA engines).  No GPSIMD ap_gather (which costs ~30-100
  Q7 cycles per index) anywhere.

  Per iteration:
    C phase (cm): msg -> t=tanh(.5 msg), sg=sign, la=ln|t|+eps, csum/cprod
      over each check's 6 edges, d=csum-la, ext=-ln(max(tanh(-d/2),TCLIP))
      * sg*cprod.  Activations phase-batched so the ACT engine only swaps
      function tables 4x per iteration (tanh and ln live in different sets).
    w1: ext_cm -> stage1 (strided DMA);  gv: gather stage1 -> ext_vm.
    A phase (vm): out = e0+e1+e2+x -> out5[it] (HBM);  msg_vm = out - ext.
    w2: msg_vm -> stage2;  gc: gather stage2 -> msg_cm (next iteration).
  Iteration 0 gathers x[var(e)] rows directly from the transposed-LLR input.

  fp16 is used for the bounded intermediate tensors (t, sign, ln|t|, csum,
  d) for SBUF capacity + DVE 2x mode; the clip chain (tanh(-d/2) in
  [TCLIP, 1], TCLIP~5e-8) must stay fp32.

  Host side: llr is transposed (and fp16-cast) before upload; the output
  [5, 8192, 64] per core is transposed back — host work is not on the
  device critical path.
"""

import os
import sys

import numpy as np

for _p in ("/opt/trn_rl_repo", "/root/.axon_site/_ro/trn_rl_repo"):
    if os.path.isdir(_p) and _p not in sys.path:
        sys.path.insert(0, _p)

N_VAR, N_CHK, DV, DC = 8192, 4096, 3, 6
E = N_VAR * DV  # 24576
BATCH, N_ITER, N_CORES = 512, int(os.environ.get("K2_ITERS", "5")), 8
BC = BATCH // N_CORES        # 64 batch rows per core
SL = E // 128                # 192 slots per partition
HSL = SL // 2                # 96 slots per compute half
DSL = SL // 4                # 48 slots per DMA chunk
TOK = 128 * DSL              # 6144 tokens per gather chunk
CHK_H = HSL // DC            # 16 checks per half per partition
VH = 32                      # vars per half per partition (64/2)

EPS = 1e-12
_C = np.float32(1.0) - np.float32(1e-7)
TCLIP = float(np.float32((np.float32(1.0) - _C) / (np.float32(1.0) + _C)))

_CACHE = {}
_LAST_RESULTS = None


def _wrap_tokens(u):
    """u: [E] token-order values -> wrapped idx plane [128, E//16] i16.

    Gathers are issued in 4 chunks of TOK tokens; each chunk's indices are
    wrapped independently: within chunk m, token i' sits at
    [p, 384*m + i'//16] for p%16 == i'%16 (replicated across the 8 cores).
    """
    u = np.asarray(u, np.int64)
    out = np.zeros((128, E // 16), np.int16)
    for m in range(4):
        c = u[m * TOK:(m + 1) * TOK].reshape(-1, 16).T  # [16, 384]
        for g in range(8):
            out[16 * g:16 * (g + 1), 384 * m:384 * (m + 1)] = c
    return out


def _build_indices(vi, ci):
    """Host-side graph preprocessing -> three wrapped index planes.

    Token i of a gather lands at SBUF (p=i%128, slot=i//128) within its
    chunk; chunk m covers slots [48m, 48m+48).
    stage rows:  cm row r1(p,j) = 192p + j;  vm row r2(v,s) = 192*(v//64)
    + 3*(v%64) + s.
    """
    order = np.argsort(ci, kind="stable")          # cm position q -> edge id
    pos_of_edge = np.empty(E, np.int64)
    pos_of_edge[order] = np.arange(E)
    edges_of_var = np.argsort(vi, kind="stable").reshape(N_VAR, DV)
    cm_var = vi[order].astype(np.int64)            # var of cm position q

    # token i -> (p, slot j) -> global position q = 192p + j
    i = np.arange(E)
    p = i % 128
    j = i // 128
    jc = j % DSL
    m = j // DSL
    jj = m * DSL + jc                              # == j
    q = 192 * p + jj

    u_x = cm_var[q]                                # x-row gather (iter 0)

    # vm slot of each edge
    v_of = cm_var                                  # var of cm position
    slot_in_var = np.zeros(E, np.int64)
    for s in range(DV):
        slot_in_var[pos_of_edge[edges_of_var[:, s]]] = s
    r2_of_q = 192 * (v_of // 64) + 3 * (v_of % 64) + slot_in_var
    u_v2c = r2_of_q[q]                             # msg_cm <- stage2(vm rows)

    # vm token i -> (p, j): v = 64p + j//3, s = j%3 -> cm row of that edge
    v = 64 * p + j // DV
    s = j % DV
    u_c2v = pos_of_edge[edges_of_var[v, s]]        # ext_vm <- stage1(cm rows)

    return {
        "icv": _wrap_tokens(u_c2v),
        "ivc": _wrap_tokens(u_v2c),
        "xsmap": vi[order].astype(np.int64),
    }


def _build_bass():
    import concourse.bass as bass  # noqa: F401  (side-effect imports)
    from concourse import bacc, mybir

    STOP = int(os.environ.get("K2_STOP", "10"))
    DBG = os.environ.get("K2_DEBUG", "0") == "1"
    DBGIT = int(os.environ.get("K2_DBGIT", "0"))

    dt = mybir.dt
    F32, F16, I16 = dt.float32, dt.float16, dt.int16
    ALU = mybir.AluOpType
    ACT = mybir.ActivationFunctionType

    nc = bacc.Bacc("TRN2", target_bir_lowering=False, debug=False)

    xs_d = nc.dram_tensor("xs", [E, BC], F32, kind="ExternalInput").ap()
    x16_d = nc.dram_tensor("x16", [N_VAR, BC], F16, kind="ExternalInput").ap()
    icv_d = nc.dram_tensor("icv", [128, E // 16], I16,
                           kind="ExternalInput").ap()
    ivc_d = nc.dram_tensor("ivc", [128, E // 16], I16,
                           kind="ExternalInput").ap()
    out_d = nc.dram_tensor("out", [N_ITER, N_VAR, BC], F32,
                           kind="ExternalOutput").ap()
    stg1_d = nc.dram_tensor("stg1", [E, BC], F32).ap()
    dbg = {}
    stg2_d = nc.dram_tensor("stg2", [E, BC], F32).ap()

    bufA = nc.alloc_sbuf_tensor("bufA", [128, SL, BC], F32).ap()
    bufB = nc.alloc_sbuf_tensor("bufB", [128, SL, BC], F32).ap()
    xsb = nc.alloc_sbuf_tensor("xsb", [128, 64, BC], F16).ap()
    ixcv = nc.alloc_sbuf_tensor("ixcv", [128, E // 16], I16).ap()
    ixvc = nc.alloc_sbuf_tensor("ixvc", [128, E // 16], I16).ap()
    BF16 = dt.bfloat16
    QF = HSL * BC // 2                 # 3072 f32 per quarter per partition
    CHK_Q = CHK_H // 2                 # 8 checks per quarter
    X1 = nc.alloc_sbuf_tensor("X1", [128, 4, QF], BF16).ap()
    X2 = nc.alloc_sbuf_tensor("X2", [128, 4, QF], BF16).ap()
    W = nc.alloc_sbuf_tensor("W", [128, 2, QF], F32).ap()
    CS = nc.alloc_sbuf_tensor("CS", [128, CHK_Q, BC], F32).ap()
    TS1 = nc.alloc_sbuf_tensor("TS1", [128, CHK_Q, BC], F32).ap()
    CP = nc.alloc_sbuf_tensor("CP", [128, 4, CHK_Q, BC], BF16).ap()
    TP1 = nc.alloc_sbuf_tensor("TP1", [128, CHK_Q, BC], BF16).ap()
    epsb = nc.alloc_sbuf_tensor("epsb", [128, 1], F32).ap()
    oneb = nc.alloc_sbuf_tensor("oneb", [128, 1], F32).ap()
    onepb = nc.alloc_sbuf_tensor("onepb", [128, 1], F32).ap()
    ov = nc.alloc_sbuf_tensor("ov", [128, VH, BC], F32).ap()

    stg1v = stg1_d.rearrange("(p r) b -> p r b", p=128)
    stg2v = stg2_d.rearrange("(p r) b -> p r b", p=128)

    sems = {}
    marks = {}

    def sem(name):
        if name not in sems:
            sems[name] = nc.alloc_semaphore(name)
        return sems[name]

    def wait(eng, name, val):
        if val <= 0:
            return
        key = (id(eng), name)
        if marks.get(key, -1) >= val:
            return
        eng.wait_ge(sem(name), val)
        marks[key] = val

    ACTE, DVE, GPS, SP = nc.scalar, nc.vector, nc.gpsimd, nc.sync

    # ---- prologue ----
    SP.dma_start(ixcv, icv_d).then_inc(sem("ld"), 16)
    SP.dma_start(ixvc, ivc_d).then_inc(sem("ld"), 16)
    SP.dma_start(xsb, x16_d.rearrange("(p v) b -> p v b", p=128)
                 ).then_inc(sem("ld"), 16)
    DVE.memset(epsb, EPS)
    DVE.memset(oneb, 1.0)
    DVE.memset(onepb, 1.0 + 1e-7).then_inc(sem("epsv"), 1)

    wait(GPS, "ld", 48)
    xsv = xs_d.rearrange("(p r) b -> p r b", p=128)
    for m in range(4):
        SP.dma_start(bufA[:, DSL * m:DSL * (m + 1), :],
                     xsv[:, DSL * m:DSL * (m + 1), :]
                     ).then_inc(sem(f"gc{m}"), 16)

    def half_views(buf, h):
        sl = buf[:, HSL * h:HSL * (h + 1), :]
        return (sl.rearrange("p r b -> p (r b)"),
                sl.rearrange("p (c t) b -> p c t b", t=DC))

    for it in range(N_ITER):
        gcb = 64 * it
        # ---- C phase: 4 quarters; PASS-A all in natural_log_exp set,
        # PASS-B tanh (set E), PASS-C ln (set L): 2 table switches/iter.
        # la = ln(tanh(|m|/2)+~eps) = ln(1-y+eps)-ln(1+y), y=e^-|m| (the
        # 1e-5-rel Exp table only feeds la, where it is harmless); the
        # magnitude tail runs through the 6e-8-rel Tanh table:
        # ext = -ln(max(tanh(-d/2), TCLIP)) * sg * cprod.
        # bf16 stores la/d/t2c/sg (floating exponent: no subnormal flush);
        # csum stays f32 so d = csum - la_e cancels la_e's stored bits.
        for q in range(4):
            n = 4 * it + q + 1
            mvq = bufA[:, DSL * q:DSL * (q + 1), :].rearrange(
                "p r b -> p (r b)")
            w = W[:, q % 2, :]
            x1 = X1[:, q, :]
            x2 = X2[:, q, :]
            la = x1.rearrange("p (c t b) -> p c t b", t=DC, b=BC)
            sg = x2.rearrange("p (c t b) -> p c t b", t=DC, b=BC)
            # -- PASS-A (set L): am, y, l1, l2, la, sg, trees, d, q-prod --
            if DBG and it == DBGIT:
                if "msg" not in dbg:
                    dbg["msg"] = nc.dram_tensor(
                        "dbg_msg", [4, 128, QF], F32,
                        kind="ExternalOutput").ap()
                wait(SP, f"gc{q}", 16 * (it + 1))
                SP.dma_start(dbg["msg"][q], mvq).then_inc(sem("dbgM"), 16)
            wait(ACTE, f"gc{q}", 16 * (it + 1))
            wait(ACTE, "ext", 4 * it)      # X2 WAR vs prev iter ext
            if q >= 2:
                wait(ACTE, "dla", 4 * it + q - 1)  # W WAR vs q-2 la read
            ACTE.activation(w, mvq, ACT.Abs).then_inc(sem("aam"), 1)
            wait(ACTE, "aam", n)
            ACTE.activation(w, w, ACT.Exp, scale=-1.0).then_inc(sem("ay"), 1)
            wait(ACTE, "ay", n)
            wait(ACTE, "epsv", 1)
            ACTE.activation(x2, w, ACT.Ln, scale=-1.0,
                            bias=onepb).then_inc(sem("al1"), 1)
            wait(ACTE, "al1", n)
            ACTE.activation(w, w, ACT.Ln, bias=oneb).then_inc(sem("al2"), 1)
            wait(DVE, "al2", n)
            DVE.tensor_tensor(x1, x2, w,
                              op=ALU.subtract).then_inc(sem("dla"), 1)
            if DBG and it == DBGIT:
                if "la" not in dbg:
                    dbg["la"] = nc.dram_tensor(
                        "dbg_la", [4, 128, QF], dt.bfloat16,
                        kind="ExternalOutput").ap()
                wait(SP, "dla", n)
                SP.dma_start(dbg["la"][q], x1).then_inc(sem("dbgA"), 16)
            wait(ACTE, "dla", n)
            ACTE.activation(x2, mvq, ACT.Sign).then_inc(sem("asg"), 1)
            rt = 10 * (4 * it + q)
            cp = CP[:, q, :, :]
            wait(DVE, "asg", n)
            wait(DVE, "dved", n - 1)       # CS/TS1 WAR vs prev quarter d
            DVE.tensor_tensor(CS, la[:, :, 0, :], la[:, :, 1, :],
                              op=ALU.add).then_inc(sem("rt"), 1)
            DVE.tensor_tensor(TS1, la[:, :, 2, :], la[:, :, 3, :],
                              op=ALU.add).then_inc(sem("rt"), 1)
            DVE.tensor_tensor(cp, sg[:, :, 0, :], sg[:, :, 1, :],
                              op=ALU.mult).then_inc(sem("rt"), 1)
            DVE.tensor_tensor(TP1, sg[:, :, 2, :], sg[:, :, 3, :],
                              op=ALU.mult).then_inc(sem("rt"), 1)
            wait(DVE, "rt", rt + 4)
            DVE.tensor_tensor(CS, CS, TS1, op=ALU.add).then_inc(sem("rt"), 1)
            DVE.tensor_tensor(cp, cp, TP1,
                              op=ALU.mult).then_inc(sem("rt"), 1)
            wait(DVE, "rt", rt + 6)
            DVE.tensor_tensor(TS1, la[:, :, 4, :], la[:, :, 5, :],
                              op=ALU.add).then_inc(sem("rt"), 1)
            DVE.tensor_tensor(TP1, sg[:, :, 4, :], sg[:, :, 5, :],
                              op=ALU.mult).then_inc(sem("rt"), 1)
            wait(DVE, "rt", rt + 8)
            DVE.tensor_tensor(CS, CS, TS1, op=ALU.add).then_inc(sem("rt"), 1)
            DVE.tensor_tensor(cp, cp, TP1,
                              op=ALU.mult).then_inc(sem("rt"), 1)
            wait(DVE, "rt", rt + 10)
            if DBG and it == DBGIT:
                wait(DVE, "dbgA", 16 * (q + 1))
            csb = CS.unsqueeze(2).broadcast_to([128, CHK_Q, DC, BC])
            DVE.tensor_tensor(la, csb, la,
                              op=ALU.subtract).then_inc(sem("dved"), 1)
            cpb = cp.unsqueeze(2).broadcast_to([128, CHK_Q, DC, BC])
            DVE.tensor_tensor(sg, sg, cpb,
                              op=ALU.mult).then_inc(sem("dveq"), 1)
            if DBG and it == DBGIT:
                if "d" not in dbg:
                    dbg["d"] = nc.dram_tensor(
                        "dbg_d", [4, 128, QF], dt.bfloat16,
                        kind="ExternalOutput").ap()
                    dbg["q"] = nc.dram_tensor(
                        "dbg_q", [4, 128, QF], dt.bfloat16,
                        kind="ExternalOutput").ap()
                wait(SP, "dved", n)
                SP.dma_start(dbg["d"][q], x1).then_inc(sem("dbgB"), 16)
                wait(SP, "dveq", n)
                SP.dma_start(dbg["q"][q], x2).then_inc(sem("dbgE"), 16)
        # -- PASS-B (set E): t2 = tanh(-d/2), clip on DVE (bf16 2x) --
        for q in range(4):
            x1 = X1[:, q, :]
            wait(ACTE, "dved", 4 * it + q + 1)
            if DBG and it == DBGIT:
                wait(ACTE, "dbgB", 16 * (q + 1))
            ACTE.activation(x1, x1, ACT.Tanh,
                            scale=-0.5).then_inc(sem("actu"), 1)
            wait(DVE, "actu", 4 * it + q + 1)
            DVE.tensor_scalar_max(x1, x1, TCLIP).then_inc(sem("gmx"), 1)
        if DBG and it == DBGIT:
            if "t2c" not in dbg:
                dbg["t2c"] = nc.dram_tensor(
                    "dbg_t2c", [128, 4, QF], dt.bfloat16,
                    kind="ExternalOutput").ap()
            wait(SP, "gmx", 4 * it + 4)
            SP.dma_start(dbg["t2c"],
                         X1.rearrange("p q f -> p (q f)")
                         ).then_inc(sem("dbgC"), 16)
        # -- PASS-C (set L): V = ln(t2c) in f32 (W); ext = (-V)*sg*cprod --
        for q in range(4):
            x1 = X1[:, q, :]
            w = W[:, q % 2, :]
            wait(ACTE, "gmx", 4 * it + q + 1)
            if DBG and it == DBGIT:
                wait(ACTE, "dbgC", 16)
            if q >= 2:
                wait(ACTE, "ext", 4 * it + q - 1)  # W WAR vs ext q-2
            ACTE.activation(w, x1, ACT.Ln).then_inc(sem("actv"), 1)
            extq = bufB[:, DSL * q:DSL * (q + 1), :].rearrange(
                "p r b -> p (r b)")
            wait(DVE, "actv", 4 * it + q + 1)
            wait(DVE, "dveq", 4 * it + q + 1)
            DVE.scalar_tensor_tensor(extq, w, -1.0, X2[:, q, :],
                                     op0=ALU.mult,
                                     op1=ALU.mult).then_inc(sem("ext"), 1)
        if STOP < 7:
            break
        # ---- ext_cm -> stage1 ----
        for m in range(4):
            wait(SP, "ext", 4 * it + m + 1)
            SP.dma_start(stg1v[:, DSL * m:DSL * (m + 1), :],
                         bufB[:, DSL * m:DSL * (m + 1), :]
                         ).then_inc(sem("w1"), 16)
        if STOP < 8:
            break
        wait(GPS, "w1", 64 * (it + 1))
        for m in range(4):
            GPS.dma_gather(bufA[:, DSL * m:DSL * (m + 1), :], stg1_d,
                           ixcv[:, 384 * m:384 * (m + 1)], TOK, TOK, BC,
                           single_packet=False).then_inc(sem(f"gv{m}"), 16)
        if STOP < 9:
            break
        # ---- A phase (vm): out rows + next msg ----
        for jh in range(2):
            ev = bufA[:, HSL * jh:HSL * (jh + 1), :].rearrange(
                "p (v s) b -> p v s b", s=DV)
            ovj = ov
            wait(DVE, f"gv{2 * jh}", 16 * (it + 1))
            wait(DVE, f"gv{2 * jh + 1}", 16 * (it + 1))
            if jh == 0:
                wait(DVE, "ow1", 16 * it)      # ov WAR vs prev iter half 1
                wait(DVE, "msg", 2 * it)       # ov WAR vs prev iter msg
            else:
                wait(DVE, "ow0", 16 * (it + 1))  # ov WAR vs half 0 this iter
                if it < N_ITER - 1:
                    wait(DVE, "msg", 2 * it + 1)  # ov WAR vs msg half 0
            DVE.tensor_tensor(ovj, ev[:, :, 0, :], ev[:, :, 1, :],
                              op=ALU.add).then_inc(sem("av1"), 1)
            wait(DVE, "av1", 2 * it + jh + 1)
            DVE.tensor_tensor(ovj, ovj, ev[:, :, 2, :],
                              op=ALU.add).then_inc(sem("av2"), 1)
            wait(DVE, "av2", 2 * it + jh + 1)
            DVE.tensor_tensor(ovj, ovj, xsb[:, VH * jh:VH * (jh + 1), :],
                              op=ALU.add).then_inc(sem("dvo"), 1)
            wait(SP, "dvo", 2 * it + jh + 1)
            SP.dma_start(
                out_d[it].rearrange("(p v) b -> p v b", p=128)
                [:, VH * jh:VH * (jh + 1), :],
                ovj).then_inc(sem(f"ow{jh}"), 16)
            if it < N_ITER - 1:
                wait(DVE, "dvo", 2 * it + jh + 1)
                ovb = ovj.unsqueeze(2).broadcast_to([128, VH, DV, BC])
                mgv = bufB[:, HSL * jh:HSL * (jh + 1), :].rearrange(
                    "p (v s) b -> p v s b", s=DV)
                DVE.tensor_tensor(mgv, ovb, ev,
                                  op=ALU.subtract).then_inc(sem("msg"), 1)
        if STOP < 10:
            break
        # ---- msg_vm -> stage2, gather -> msg_cm ----
        if it < N_ITER - 1:
            for m in range(2):
                wait(SP, "msg", 2 * it + m + 1)
                SP.dma_start(stg2v[:, HSL * m:HSL * (m + 1), :],
                             bufB[:, HSL * m:HSL * (m + 1), :]
                             ).then_inc(sem("w2"), 16)
            wait(GPS, "w2", 32 * (it + 1))
            for m in range(4):
                GPS.dma_gather(bufA[:, DSL * m:DSL * (m + 1), :], stg2_d,
                               ixvc[:, 384 * m:384 * (m + 1)], TOK, TOK, BC,
                               single_packet=False).then_inc(sem(f"gc{m}"), 16)

    ni = N_ITER if STOP >= 10 else 1
    if STOP >= 9:
        wait(SP, "ow0", 16 * ni)
        wait(SP, "ow1", 16 * ni)
    elif STOP >= 8:
        wait(SP, "gv0", 32 * ni)
        wait(SP, "gv1", 32 * ni)
    elif STOP >= 7:
        wait(SP, "w1", 32 * ni)
    elif STOP >= 6:
        wait(SP, "ext", 2 * ni)
    elif STOP >= 5:
        wait(SP, "dvem", 2 * ni)
    elif STOP >= 4:
        wait(SP, "dved", 2 * ni)
    elif STOP >= 3:
        wait(SP, "dla", 2 * ni)
    elif STOP >= 2:
        wait(SP, "al2", 2 * ni)
    else:
        for m in range(4):
            wait(SP, f"gc{m}", 16)

    nc.compile()
    return nc


def _numpy_fallback(llr, vi, ci):
    x = llr.T.astype(np.float32)
    scattered = x[vi]
    ext = np.zeros_like(scattered)
    outs = []
    for _ in range(N_ITER):
        vsum = np.zeros((N_VAR, x.shape[1]), np.float32)
        np.add.at(vsum, vi, ext)
        msg = (vsum[vi] - ext) + scattered
        t = np.tanh(msg * 0.5)
        la = np.log(np.abs(t) + EPS)
        sg = np.sign(t)
        cs = np.zeros((N_CHK, x.shape[1]), np.float32)
        np.add.at(cs, ci, la)
        cpr = np.ones((N_CHK, x.shape[1]), np.float32)
        np.multiply.at(cpr, ci, sg)
        loo = np.exp(cs[ci] - la) * (cpr[ci] * sg)
        loo = np.clip(loo, -float(_C), float(_C))
        ext = 2.0 * np.arctanh(loo)
        vs2 = np.zeros((N_VAR, x.shape[1]), np.float32)
        np.add.at(vs2, vi, ext)
        outs.append((vs2 + x).T)
    return np.stack(outs)


def _core_inputs(llr, planes, c):
    xt = np.ascontiguousarray(llr[c * BC:(c + 1) * BC, :].T, np.float32)
    m = {nm: np.ascontiguousarray(v) for nm, v in planes.items()
         if nm != "xsmap"}
    m["xs"] = np.ascontiguousarray(xt[planes["xsmap"]])
    m["x16"] = xt.astype(np.float16)
    return m


def kernel(llr, var_index, chk_index):
    llr = np.asarray(llr, np.float32)
    vi = np.asarray(var_index, np.int64).ravel()
    ci = np.asarray(chk_index, np.int64).ravel()
    assert llr.shape == (BATCH, N_VAR) and vi.shape == (E,) and ci.shape == (E,)

    regular = (np.array_equal(np.bincount(vi, minlength=N_VAR),
                              np.full(N_VAR, DV))
               and np.array_equal(np.bincount(ci, minlength=N_CHK),
                                  np.full(N_CHK, DC)))
    if not regular:
        return _numpy_fallback(llr, vi, ci).astype(np.float32)

    key = ("k2", hash(vi.tobytes()), hash(ci.tobytes()))
    if key not in _CACHE:
        planes = _build_indices(vi, ci)
        nc = _build_bass()
        _CACHE[key] = (nc, planes)
    nc, planes = _CACHE[key]

    from concourse.bass_utils import run_bass_kernel_spmd
    in_maps = [_core_inputs(llr, planes, c) for c in range(N_CORES)]
    trace = os.environ.get("BASS_KERNEL_TRACE", "0") == "1"
    res = run_bass_kernel_spmd(nc, in_maps, list(range(N_CORES)), trace=trace)
    global _LAST_RESULTS
    _LAST_RESULTS = res
    out = np.concatenate(
        [np.transpose(res.results[c]["out"], (0, 2, 1))
         for c in range(N_CORES)], axis=1)
    return np.ascontiguousarray(out, dtype=np.float32)


if __name__ == "__main__":
    sys.path.insert(0, os.path.dirname(os.path.abspath(__file__)))
    import reference
    inputs = {k: np.asarray(v) for k, v in reference.setup_inputs().items()}
    llr = np.asarray(inputs["llr"], np.float32)
    vi = np.asarray(inputs["var_index"], np.int64)
    ci = np.asarray(inputs["chk_index"], np.int64)
    exp = _numpy_fallback(llr, vi, ci)

    if "--sim" in sys.argv:
        from concourse.bass_interp import CoreSim
        planes = _build_indices(vi, ci)
        nc = _build_bass()
        sim = CoreSim(nc)
        for k, v in _core_inputs(llr, planes, 0).items():
            sim.tensor(k)[:] = v
        sim.simulate()
        got0 = np.transpose(np.asarray(sim.tensor("out")), (0, 2, 1))
        exp0 = exp[:, 0:BC, :]
        err = np.max(np.abs(got0 - exp0)) / (np.max(np.abs(exp0)) + 1e-30)
        print("SIM relative error (core 0):", err)
    else:
        got = kernel(**inputs)
        err = np.max(np.abs(got - exp)) / (np.max(np.abs(exp)) + 1e-30)
        print("Relative error:", err)
